# revision 30
# baseline (speedup 1.0000x reference)
"""MultiHeadAttention TRN2 kernel v2: B=2, S=2048, D=1024, H=16, Dh=64.

Sharding (8 cores): core c -> batch b=c//4, head-group g=c%4 (4 heads = 2
head-pairs, 256 model dims).  Tensor-parallel QKV + row-parallel output
projection; 4-way partial-output sum + bo + bv@Wo on host.

fp16 on-chip pipeline (host-sim rel err 1.2e-3):
  Q^T,K^T [128,2,2048] fp16   (dh+64*(h%2) on partitions, pair ht on free)
  V       [128,kt,h,64] fp16  natural (token on partitions)
  scores  s^T[k,q]: 2-head ROW-TILED matmul pairs (64x128 tiles), psum f32
  pt      fp16 raw scores staged to SBUF (DVE evac), exp'd IN PLACE by ACT
          in [128,4096] calls (scale=0.125)
  denom   4-head COL-TILED (128x32) ones-matmuls -> psum partitions 32h
  ctx     2-head COL-TILED (128x64) matmuls, accum over kt
  recip   DVE reciprocal on full [128,512] psum (memset-1.0 background)
  bcast   selector-matmul E^T @ recip -> per-q recip rows, DVE normalize
  out     ctx^T @ Wo natural, f32 partials to HBM

Emission is software-pipelined so ACT (the bottleneck: 16.8M exps/core)
never starves: fill = K-proj, Q0-proj, S(0), V-proj, exp(0), Q1-proj; then
per chunk: D, C, bcast+norm, Q-proj(c+2), O-proj(c), S(c+1), exp(c+1).
"""

import numpy as np

import concourse.bass as bass
import concourse.mybir as mybir
import concourse.tile as tile_mod
from concourse.tile import TileContext
from concourse.bass_utils import run_bass_kernel_spmd
from concourse.vector_clock import ScopedClock

# ---------------------------------------------------------------- drain patch
# This walrus build's TPB_CTRL drain lowering accepts only ONE sync wait per
# instruction; TileContext's tail drain carries one wait per live semaphore.
# Split it into a chain of drains with <=1 wait each.
_MAXW = 1


def _patched_drain_and_barrier(self, tick_clock, wait_clock):
    nc = self.nc
    drain_inst = nc.sync.drain()
    wait_clock.add_sem_waits(
        drain_inst.ins, ScopedClock({None: tick_clock.global_clock})
    )
    si = drain_inst.ins.sync_info
    if si is not None and si.on_wait and len(si.on_wait) > _MAXW:
        waits = list(si.on_wait)
        del si.on_wait[_MAXW:]
        for i in range(_MAXW, len(waits), _MAXW):
            d2 = nc.sync.drain()
            si2 = d2.ins.sync_info
            if si2 is None:
                d2.ins.sync_info = mybir.SyncInfo(on_wait=[], on_update=[])
                si2 = d2.ins.sync_info
            si2.on_wait.extend(waits[i : i + _MAXW])
    nc.all_engine_barrier()
    assert self.sems is not None
    popped = nc._tile_sem_poison_stack.pop()
    assert popped is self._sem_poison
    nc.clear_and_free_semaphores(list(self.sems.allocated().values()))
    nc.all_engine_barrier()


tile_mod.TileContext._drain_and_barrier = _patched_drain_and_barrier

# ---------------------------------------------------------------- constants
B, S, D = 2, 2048, 1024
H, DH = 16, 64
N_CORES = 8
HPC = 4          # heads per core (2 pairs)
GD = HPC * DH    # 256 model dims per core
KT = S // 128    # 16 key tiles
NCH = 4          # q chunks of 512
F32 = mybir.dt.float32
F16 = mybir.dt.float16


def _split_excess_waits(nc):
    """This walrus build accepts only ONE sync wait per instruction (any
    type).  Hoist extra waits onto same-engine nops inserted right before
    the over-subscribed instruction."""
    for fn in nc.m.functions:
        for bb in fn.blocks:
            insts = bb.instructions
            i = 0
            while i < len(insts):
                inst = insts[i]
                si = getattr(inst, "sync_info", None)
                if si is not None and si.on_wait and len(si.on_wait) > 1:
                    extra = list(si.on_wait[:-1])
                    del si.on_wait[:-1]
                    nops = []
                    for w in extra:
                        bi = nc.engines[inst.engine].nop(nofuse=True,
                                                         hint="waitsplit")
                        bi.ins.sync_info = mybir.SyncInfo(on_wait=[w],
                                                          on_update=[])
                        nops.append(bi.ins)
                    for ni in nops:
                        for fb in fn.blocks:
                            if ni in fb.instructions:
                                fb.instructions.remove(ni)
                                break
                    insts[i:i] = nops
                    i += len(nops)
                i += 1


def _build():
    from contextlib import ExitStack

    nc = bass.Bass("TRN2", target_bir_lowering=False, debug=False,
                   num_devices=N_CORES)
    d_xqT = nc.dram_tensor("xqT", [D, S], F16, kind="ExternalInput").ap()
    d_xkT = nc.dram_tensor("xkT", [D, S], F16, kind="ExternalInput").ap()
    d_xvT = nc.dram_tensor("xvT", [D, S], F16, kind="ExternalInput").ap()
    d_wq = nc.dram_tensor("wq", [D, GD], F16, kind="ExternalInput").ap()
    d_wk = nc.dram_tensor("wk", [D, GD], F16, kind="ExternalInput").ap()
    d_wv = nc.dram_tensor("wv", [D, GD], F16, kind="ExternalInput").ap()
    d_wo = nc.dram_tensor("wo", [GD, D], F16, kind="ExternalInput").ap()
    d_bq = nc.dram_tensor("bq", [GD], F32, kind="ExternalInput").ap()
    d_bk = nc.dram_tensor("bk", [GD], F32, kind="ExternalInput").ap()
    d_out = nc.dram_tensor("out", [S, D], F32, kind="ExternalOutput").ap()

    with TileContext(nc) as tc, ExitStack() as ctx:
        ctx.enter_context(nc.allow_low_precision(
            reason="fp16 on-chip pipeline; accumulation stays fp32 in PSUM"))
        wp = ctx.enter_context(tc.tile_pool(name="w", bufs=1))
        xp = ctx.enter_context(tc.tile_pool(name="x", bufs=2))
        big = ctx.enter_context(tc.tile_pool(name="big", bufs=1))
        outp = ctx.enter_context(tc.tile_pool(name="outp", bufs=2))
        misc = ctx.enter_context(tc.tile_pool(name="misc", bufs=2))
        ps_s = ctx.enter_context(
            tc.tile_pool(name="ps_s", bufs=2, space="PSUM"))
        ps_c = ctx.enter_context(
            tc.tile_pool(name="ps_c", bufs=2, space="PSUM"))
        ps_d = ctx.enter_context(
            tc.tile_pool(name="ps_d", bufs=1, space="PSUM"))
        ps_b = ctx.enter_context(
            tc.tile_pool(name="ps_b", bufs=1, space="PSUM"))

        # ---- weights to SBUF (k-tiled layouts); wq/bq first so the Q0
        # projection (the first compute) is unblocked earliest.
        wq_sb = wp.tile([128, 8, GD], F16, tag="wq")
        nc.sync.dma_start(out=wq_sb, in_=d_wq.rearrange("(k p) n -> p k n", p=128))
        bq_sb = wp.tile([128, 2], F32, tag="bq")
        nc.sync.dma_start(out=bq_sb, in_=d_bq.rearrange("(m p) -> p m", p=128))
        wk_sb = wp.tile([128, 8, GD], F16, tag="wk")
        bk_sb = wp.tile([128, 2], F32, tag="bk")
        wv_sb = wp.tile([128, 8, GD], F16, tag="wv")
        wo_sb = wp.tile([128, 2, D], F16, tag="wo")

        def load_late_weights():
            # emitted after Q0's input DMA so the first projection isn't
            # queued behind 1.5MB of weights it doesn't need yet
            nc.sync.dma_start(out=wk_sb,
                              in_=d_wk.rearrange("(k p) n -> p k n", p=128))
            nc.sync.dma_start(out=bk_sb,
                              in_=d_bk.rearrange("(m p) -> p m", p=128))
            nc.sync.dma_start(out=wv_sb,
                              in_=d_wv.rearrange("(k p) n -> p k n", p=128))
            nc.sync.dma_start(out=wo_sb,
                              in_=d_wo.rearrange("(k p) n -> p k n", p=128))

        # ones column for denominator matmuls
        ones_sb = wp.tile([128, 1], F16, tag="ones")
        nc.vector.memset(ones_sb, 1.0)
        # bcast selector: E[:, p, :]^T @ recip replicates denominator-recip
        # rows {64j} -> partition blocks [64j:64j+64) for pair p's heads.
        E_sb = wp.tile([128, 2, 128], F16, tag="E")
        nc.vector.memset(E_sb, 0.0)
        for p in range(2):
            for j in range(2):
                nc.vector.memset(E_sb[32 * (2 * p + j):32 * (2 * p + j) + 1,
                                      p, 64 * j:64 * j + 64], 1.0)

        qt_sb = big.tile([128, 2, S], F16, tag="qt")
        kt_sb = big.tile([128, 2, S], F16, tag="kt")
        vp_sb = big.tile([128, KT, HPC, DH], F16, tag="vp")
        ctxT_sb = big.tile([128, 2, S], F16, tag="ctxT")
        pt_bufs = [big.tile([128, KT, HPC, 512], F16, tag=f"pt{i}",
                            name=f"pt{i}") for i in range(2)]
        recip_sb = big.tile([128, 512], F16, tag="recip")

        # ---- projection helpers -------------------------------------------
        def proj_T(d_x, w_sb, b_sb, dst, n):
            """(x @ W + b)^T for one 512-token chunk -> dst[:, :, n*512:...]"""
            xr = d_x.rearrange("(k p) q -> p k q", p=128)
            xb = xp.tile([128, 8, 512], F16, tag="xb")
            nc.sync.dma_start(out=xb, in_=xr[:, :, n * 512:(n + 1) * 512])
            for m in range(2):
                ps = ps_c.tile([128, 512], F32, tag="pc")
                for k in range(8):
                    nc.tensor.matmul(ps, w_sb[:, k, m * 128:(m + 1) * 128],
                                     xb[:, k, :], start=(k == 0), stop=(k == 7))
                nc.vector.tensor_scalar_add(
                    dst[:, m, n * 512:(n + 1) * 512], ps, b_sb[:, m:m + 1])

        def proj_V(n):
            """V natural [tok,256] for one 512-token chunk (no bias: bv@Wo
            is folded into the host-side output bias)."""
            xr = d_xvT.rearrange("(k p) q -> p k q", p=128)
            xb = xp.tile([128, 8, 512], F16, tag="xb")
            nc.sync.dma_start(out=xb, in_=xr[:, :, n * 512:(n + 1) * 512])
            for t in range(4):
                ps = ps_b.tile([128, 512], F32, tag="bc", name="v_ps")
                for k in range(8):
                    nc.tensor.matmul(ps[:, 0:GD],
                                     xb[:, k, t * 128:(t + 1) * 128],
                                     wv_sb[:, k, :],
                                     start=(k == 0), stop=(k == 7))
                nc.vector.tensor_copy(
                    vp_sb[:, n * 4 + t, :, :],
                    ps[:, 0:GD].rearrange("p (h d) -> p h d", h=HPC))

        # kt pairs whose exp runs on ACT directly from PSUM (skipping the
        # DVE cast) -- balances ACT vs DVE throughput per cadence.
        DIRECT = set(range(16))

        # ---- attention phase helpers --------------------------------------
        def scores_block(c, b, direct_ps):
            """Row-tiled 2-head score matmuls for kts 4b..4b+3.  Non-DIRECT
            kts are DVE-cast to pt; DIRECT kts leave psum tiles for ACT."""
            pt_sb = pt_bufs[c % 2]
            for kt in range(4 * b, 4 * b + 4):
                for ht in range(2):
                    ps = ps_s.tile([128, 1024], F32, tag="s", name="s_ps")
                    for j in range(2):
                        nc.tensor.matmul(
                            ps[:, j * 512:(j + 1) * 512],
                            kt_sb[64 * j:64 * j + 64, ht,
                                  kt * 128:(kt + 1) * 128],
                            qt_sb[64 * j:64 * j + 64, ht,
                                  c * 512:(c + 1) * 512],
                            start=True, stop=True)
                    if kt in DIRECT:
                        direct_ps[(kt, ht)] = ps
                    else:
                        nc.vector.tensor_copy(
                            pt_sb[:, kt, 2 * ht:2 * ht + 2, :],
                            ps.rearrange("p (h q) -> p h q", h=2))

        def exp_block(c, b, direct_ps):
            """ACT exp for kts 4b..4b+3: big in-place SBUF calls for the
            DVE-cast pairs, direct PSUM->SBUF calls for DIRECT kts."""
            pt_sb = pt_bufs[c % 2]
            for kt0 in (4 * b, 4 * b + 2):
                if kt0 in DIRECT:
                    for kt in (kt0, kt0 + 1):
                        for ht in range(2):
                            ps = direct_ps.pop((kt, ht))
                            nc.scalar.activation(
                                pt_sb[:, kt, 2 * ht:2 * ht + 2, :],
                                ps.rearrange("p (h q) -> p h q", h=2),
                                mybir.ActivationFunctionType.Exp, scale=0.125)
                else:
                    sl = pt_sb[:, kt0:kt0 + 2, :, :]
                    nc.scalar.activation(sl, sl,
                                         mybir.ActivationFunctionType.Exp,
                                         scale=0.125)

        def denom_block(c, ps, b):
            """Col-tiled (128x32) 4-head denominator accumulation."""
            pt_sb = pt_bufs[c % 2]
            for kt in range(4 * b, 4 * b + 4):
                for h in range(HPC):
                    nc.tensor.matmul(ps[32 * h:32 * h + 1, :], ones_sb,
                                     pt_sb[:, kt, h, :],
                                     start=(kt == 0), stop=(kt == KT - 1),
                                     skip_group_check=True,
                                     tile_position=(0, 32 * h))

        def ctx_block(c, ctx_ps, b):
            """Col-tiled (128x64) 2-head ctx accumulation for kts of b."""
            pt_sb = pt_bufs[c % 2]
            for kt in range(4 * b, 4 * b + 4):
                for h in range(HPC):
                    ht, j = h // 2, h % 2
                    nc.tensor.matmul(ctx_ps[ht][64 * j:64 * j + 64, :],
                                     vp_sb[:, kt, h, :], pt_sb[:, kt, h, :],
                                     start=(kt == 0), stop=(kt == KT - 1),
                                     skip_group_check=True)

        def finish_chunk(c, d_ps, ctx_ps):
            """reciprocal of denominators, bcast via selector matmul, then
            normalize ctx into ctxT."""
            nc.vector.reciprocal(recip_sb, d_ps)
            for ht in range(2):
                bc = ps_b.tile([128, 512], F32, tag="bc")
                nc.tensor.matmul(bc, E_sb[:, ht, :], recip_sb,
                                 start=True, stop=True)
                bc_sb = misc.tile([128, 512], F32, tag="bc_sb")
                nc.vector.tensor_copy(bc_sb, bc)
                nc.vector.tensor_mul(
                    ctxT_sb[:, ht, c * 512:(c + 1) * 512], ctx_ps[ht], bc_sb)

        def out_proj_t(c, t):
            """One 128-token tile of out = ctx^T @ Wo -> HBM (f32 partial)."""
            tok = c * 512 + t * 128
            o_sb = outp.tile([128, D], F32, tag="o")
            ps = ps_s.tile([128, 1024], F32, tag="s", name="o_ps")
            for k in range(2):
                for n in range(2):
                    nc.tensor.matmul(
                        ps[:, n * 512:(n + 1) * 512],
                        ctxT_sb[:, k, tok:tok + 128],
                        wo_sb[:, k, n * 512:(n + 1) * 512],
                        start=(k == 0), stop=(k == 1))
            nc.vector.tensor_copy(o_sb, ps)
            nc.sync.dma_start(out=d_out[tok:tok + 128, :], in_=o_sb)

        # ---- emission schedule (software-pipelined) -----------------------
        # ACT (exp) is the throughput bound; every cadence keeps it fed by
        # interleaving next-chunk scores into current-chunk consumption.
        # Prev-chunk out-projection is spread per block so no ACT-idle
        # consumption tail forms at cadence boundaries.
        direct_ps = {}
        proj_T(d_xqT, wq_sb, bq_sb, qt_sb, 0)
        load_late_weights()
        for b in range(4):
            proj_T(d_xkT, wk_sb, bk_sb, kt_sb, b)
            scores_block(0, b, direct_ps)
            exp_block(0, b, direct_ps)
        proj_T(d_xqT, wq_sb, bq_sb, qt_sb, 1)

        for c in range(NCH):
            d_ps = ps_d.tile([128, 512], F32, tag="d")
            nc.vector.memset(d_ps, 1.0)  # reciprocal background
            ctx_ps = [ps_c.tile([128, 512], F32, tag="pc", name=f"ctx{i}")
                      for i in range(2)]
            for b in range(4):
                if c == 0:
                    proj_V(b)
                if c >= 1:
                    out_proj_t(c - 1, b)
                denom_block(c, d_ps, b)
                ctx_block(c, ctx_ps, b)
                if c + 1 < NCH:
                    scores_block(c + 1, b, direct_ps)
                    exp_block(c + 1, b, direct_ps)
            finish_chunk(c, d_ps, ctx_ps)
            if c + 2 < NCH:
                proj_T(d_xqT, wq_sb, bq_sb, qt_sb, c + 2)
        for t in range(4):
            out_proj_t(NCH - 1, t)

    _split_excess_waits(nc)
    return nc


_NC = None


def _get_nc():
    global _NC
    if _NC is None:
        _NC = _build()
    return _NC


def _make_in_maps(inputs):
    query = np.asarray(inputs["query"], np.float32)
    key = np.asarray(inputs["key"], np.float32)
    value = np.asarray(inputs["value"], np.float32)
    Wq, Wk, Wv, Wo = (np.asarray(inputs[a], np.float32)
                      for a in ("Wq", "Wk", "Wv", "Wo"))
    bq, bk = (np.asarray(inputs[a], np.float32) for a in ("bq", "bk"))

    in_maps = []
    for c in range(N_CORES):
        b, g = divmod(c, HPC)
        sl = slice(g * GD, (g + 1) * GD)
        in_maps.append({
            "xqT": np.ascontiguousarray(query[b].T.astype(np.float16)),
            "xkT": np.ascontiguousarray(key[b].T.astype(np.float16)),
            "xvT": np.ascontiguousarray(value[b].T.astype(np.float16)),
            "wq": np.ascontiguousarray(Wq[:, sl].astype(np.float16)),
            "wk": np.ascontiguousarray(Wk[:, sl].astype(np.float16)),
            "wv": np.ascontiguousarray(Wv[:, sl].astype(np.float16)),
            "wo": np.ascontiguousarray(Wo[sl, :].astype(np.float16)),
            "bq": np.ascontiguousarray(bq[sl]),
            "bk": np.ascontiguousarray(bk[sl]),
        })
    return in_maps


def kernel(query, key, value, Wq, bq, Wk, bk, Wv, bv, Wo, bo):
    bo = np.asarray(bo, np.float32)
    bv = np.asarray(bv, np.float32)
    Wo_f = np.asarray(Wo, np.float32)
    in_maps = _make_in_maps(dict(query=query, key=key, value=value,
                                 Wq=Wq, bq=bq, Wk=Wk, bk=bk, Wv=Wv,
                                 Wo=Wo))

    res = run_bass_kernel_spmd(_get_nc(), in_maps, list(range(N_CORES)))
    outs = [res.results[c]["out"] for c in range(N_CORES)]
    bias = bo + bv @ Wo_f  # bv enters ctx additively (softmax weights sum to 1)
    full = np.stack([
        outs[0] + outs[1] + outs[2] + outs[3],
        outs[4] + outs[5] + outs[6] + outs[7],
    ]).astype(np.float32)
    return full + bias


# revision 31
# speedup vs baseline: 1.0020x; 1.0020x over previous
"""MultiHeadAttention TRN2 kernel v2: B=2, S=2048, D=1024, H=16, Dh=64.

Sharding (8 cores): core c -> batch b=c//4, head-group g=c%4 (4 heads = 2
head-pairs, 256 model dims).  Tensor-parallel QKV + row-parallel output
projection; 4-way partial-output sum + bo + bv@Wo on host.

fp16 on-chip pipeline (host-sim rel err 1.2e-3):
  Q^T,K^T [128,2,2048] fp16   (dh+64*(h%2) on partitions, pair ht on free)
  V       [128,kt,h,64] fp16  natural (token on partitions)
  scores  s^T[k,q]: 2-head ROW-TILED matmul pairs (64x128 tiles), psum f32
  pt      fp16 raw scores staged to SBUF (DVE evac), exp'd IN PLACE by ACT
          in [128,4096] calls (scale=0.125)
  denom   4-head COL-TILED (128x32) ones-matmuls -> psum partitions 32h
  ctx     2-head COL-TILED (128x64) matmuls, accum over kt
  recip   DVE reciprocal on full [128,512] psum (memset-1.0 background)
  bcast   selector-matmul E^T @ recip -> per-q recip rows, DVE normalize
  out     ctx^T @ Wo natural, f32 partials to HBM

Emission is software-pipelined so ACT (the bottleneck: 16.8M exps/core)
never starves: fill = K-proj, Q0-proj, S(0), V-proj, exp(0), Q1-proj; then
per chunk: D, C, bcast+norm, Q-proj(c+2), O-proj(c), S(c+1), exp(c+1).
"""

import numpy as np

import concourse.bass as bass
import concourse.mybir as mybir
import concourse.tile as tile_mod
from concourse.tile import TileContext
from concourse.bass_utils import run_bass_kernel_spmd
from concourse.vector_clock import ScopedClock

# ---------------------------------------------------------------- drain patch
# This walrus build's TPB_CTRL drain lowering accepts only ONE sync wait per
# instruction; TileContext's tail drain carries one wait per live semaphore.
# Split it into a chain of drains with <=1 wait each.
_MAXW = 1


def _patched_drain_and_barrier(self, tick_clock, wait_clock):
    nc = self.nc
    drain_inst = nc.sync.drain()
    wait_clock.add_sem_waits(
        drain_inst.ins, ScopedClock({None: tick_clock.global_clock})
    )
    si = drain_inst.ins.sync_info
    if si is not None and si.on_wait and len(si.on_wait) > _MAXW:
        waits = list(si.on_wait)
        del si.on_wait[_MAXW:]
        for i in range(_MAXW, len(waits), _MAXW):
            d2 = nc.sync.drain()
            si2 = d2.ins.sync_info
            if si2 is None:
                d2.ins.sync_info = mybir.SyncInfo(on_wait=[], on_update=[])
                si2 = d2.ins.sync_info
            si2.on_wait.extend(waits[i : i + _MAXW])
    nc.all_engine_barrier()
    assert self.sems is not None
    popped = nc._tile_sem_poison_stack.pop()
    assert popped is self._sem_poison
    nc.clear_and_free_semaphores(list(self.sems.allocated().values()))
    nc.all_engine_barrier()


tile_mod.TileContext._drain_and_barrier = _patched_drain_and_barrier

# ---------------------------------------------------------------- constants
B, S, D = 2, 2048, 1024
H, DH = 16, 64
N_CORES = 8
HPC = 4          # heads per core (2 pairs)
GD = HPC * DH    # 256 model dims per core
KT = S // 128    # 16 key tiles
NCH = 4          # q chunks of 512
F32 = mybir.dt.float32
F16 = mybir.dt.float16


def _split_excess_waits(nc):
    """This walrus build accepts only ONE sync wait per instruction (any
    type).  Hoist extra waits onto same-engine nops inserted right before
    the over-subscribed instruction."""
    for fn in nc.m.functions:
        for bb in fn.blocks:
            insts = bb.instructions
            i = 0
            while i < len(insts):
                inst = insts[i]
                si = getattr(inst, "sync_info", None)
                if si is not None and si.on_wait and len(si.on_wait) > 1:
                    extra = list(si.on_wait[:-1])
                    del si.on_wait[:-1]
                    nops = []
                    for w in extra:
                        bi = nc.engines[inst.engine].nop(nofuse=True,
                                                         hint="waitsplit")
                        bi.ins.sync_info = mybir.SyncInfo(on_wait=[w],
                                                          on_update=[])
                        nops.append(bi.ins)
                    for ni in nops:
                        for fb in fn.blocks:
                            if ni in fb.instructions:
                                fb.instructions.remove(ni)
                                break
                    insts[i:i] = nops
                    i += len(nops)
                i += 1


def _build():
    from contextlib import ExitStack

    nc = bass.Bass("TRN2", target_bir_lowering=False, debug=False,
                   num_devices=N_CORES)
    d_xqT = nc.dram_tensor("xqT", [D, S], F16, kind="ExternalInput").ap()
    d_xkT = nc.dram_tensor("xkT", [D, S], F16, kind="ExternalInput").ap()
    d_xvT = nc.dram_tensor("xvT", [D, S], F16, kind="ExternalInput").ap()
    d_wq = nc.dram_tensor("wq", [D, GD], F16, kind="ExternalInput").ap()
    d_wk = nc.dram_tensor("wk", [D, GD], F16, kind="ExternalInput").ap()
    d_wv = nc.dram_tensor("wv", [D, GD], F16, kind="ExternalInput").ap()
    d_wo = nc.dram_tensor("wo", [GD, D], F16, kind="ExternalInput").ap()
    d_bq = nc.dram_tensor("bq", [GD], F32, kind="ExternalInput").ap()
    d_bk = nc.dram_tensor("bk", [GD], F32, kind="ExternalInput").ap()
    d_out = nc.dram_tensor("out", [S, D], F32, kind="ExternalOutput").ap()

    with TileContext(nc) as tc, ExitStack() as ctx:
        ctx.enter_context(nc.allow_low_precision(
            reason="fp16 on-chip pipeline; accumulation stays fp32 in PSUM"))
        wp = ctx.enter_context(tc.tile_pool(name="w", bufs=1))
        xp = ctx.enter_context(tc.tile_pool(name="x", bufs=3))
        big = ctx.enter_context(tc.tile_pool(name="big", bufs=1))
        outp = ctx.enter_context(tc.tile_pool(name="outp", bufs=2))
        misc = ctx.enter_context(tc.tile_pool(name="misc", bufs=2))
        ps_s = ctx.enter_context(
            tc.tile_pool(name="ps_s", bufs=2, space="PSUM"))
        ps_c = ctx.enter_context(
            tc.tile_pool(name="ps_c", bufs=2, space="PSUM"))
        ps_d = ctx.enter_context(
            tc.tile_pool(name="ps_d", bufs=1, space="PSUM"))
        ps_b = ctx.enter_context(
            tc.tile_pool(name="ps_b", bufs=1, space="PSUM"))

        # ---- weights to SBUF (k-tiled layouts); wq/bq first so the Q0
        # projection (the first compute) is unblocked earliest.
        wq_sb = wp.tile([128, 8, GD], F16, tag="wq")
        nc.sync.dma_start(out=wq_sb, in_=d_wq.rearrange("(k p) n -> p k n", p=128))
        bq_sb = wp.tile([128, 2], F32, tag="bq")
        nc.sync.dma_start(out=bq_sb, in_=d_bq.rearrange("(m p) -> p m", p=128))
        wk_sb = wp.tile([128, 8, GD], F16, tag="wk")
        bk_sb = wp.tile([128, 2], F32, tag="bk")
        wv_sb = wp.tile([128, 8, GD], F16, tag="wv")
        wo_sb = wp.tile([128, 2, D], F16, tag="wo")

        def load_late_weights():
            nc.sync.dma_start(out=wk_sb,
                              in_=d_wk.rearrange("(k p) n -> p k n", p=128))
            nc.sync.dma_start(out=bk_sb,
                              in_=d_bk.rearrange("(m p) -> p m", p=128))
            nc.sync.dma_start(out=wv_sb,
                              in_=d_wv.rearrange("(k p) n -> p k n", p=128))
            nc.sync.dma_start(out=wo_sb,
                              in_=d_wo.rearrange("(k p) n -> p k n", p=128))

        # ones column for denominator matmuls
        ones_sb = wp.tile([128, 1], F16, tag="ones")
        nc.vector.memset(ones_sb, 1.0)
        # bcast selector: E[:, p, :]^T @ recip replicates denominator-recip
        # rows {64j} -> partition blocks [64j:64j+64) for pair p's heads.
        E_sb = wp.tile([128, 2, 128], F16, tag="E")
        nc.vector.memset(E_sb, 0.0)
        for p in range(2):
            for j in range(2):
                nc.vector.memset(E_sb[32 * (2 * p + j):32 * (2 * p + j) + 1,
                                      p, 64 * j:64 * j + 64], 1.0)

        qt_sb = big.tile([128, 2, S], F16, tag="qt")
        kt_sb = big.tile([128, 2, S], F16, tag="kt")
        vp_sb = big.tile([128, KT, HPC, DH], F16, tag="vp")
        ctxT_sb = big.tile([128, 2, S], F16, tag="ctxT")
        pt_sb = big.tile([128, KT, HPC, 512], F16, tag="pt")
        recip_sb = big.tile([128, 512], F16, tag="recip")

        # ---- projection helpers -------------------------------------------
        def proj_T(d_x, w_sb, b_sb, dst, n):
            """(x @ W + b)^T for one 512-token chunk -> dst[:, :, n*512:...]"""
            xr = d_x.rearrange("(k p) q -> p k q", p=128)
            xb = xp.tile([128, 8, 512], F16, tag="xb")
            nc.sync.dma_start(out=xb, in_=xr[:, :, n * 512:(n + 1) * 512])
            for m in range(2):
                ps = ps_c.tile([128, 512], F32, tag="pc")
                for k in range(8):
                    nc.tensor.matmul(ps, w_sb[:, k, m * 128:(m + 1) * 128],
                                     xb[:, k, :], start=(k == 0), stop=(k == 7))
                nc.vector.tensor_scalar_add(
                    dst[:, m, n * 512:(n + 1) * 512], ps, b_sb[:, m:m + 1])

        def proj_V(n):
            """V natural [tok,256] for one 512-token chunk (no bias: bv@Wo
            is folded into the host-side output bias)."""
            xr = d_xvT.rearrange("(k p) q -> p k q", p=128)
            xb = xp.tile([128, 8, 512], F16, tag="xb")
            nc.sync.dma_start(out=xb, in_=xr[:, :, n * 512:(n + 1) * 512])
            for t in range(4):
                ps = ps_b.tile([128, 512], F32, tag="bc", name="v_ps")
                for k in range(8):
                    nc.tensor.matmul(ps[:, 0:GD],
                                     xb[:, k, t * 128:(t + 1) * 128],
                                     wv_sb[:, k, :],
                                     start=(k == 0), stop=(k == 7))
                nc.vector.tensor_copy(
                    vp_sb[:, n * 4 + t, :, :],
                    ps[:, 0:GD].rearrange("p (h d) -> p h d", h=HPC))

        # kt pairs whose exp runs on ACT directly from PSUM (skipping the
        # DVE cast) -- balances ACT vs DVE throughput per cadence.
        DIRECT = set(range(16))

        # ---- attention phase helpers --------------------------------------
        def scores_block(c, b, direct_ps):
            """Row-tiled 2-head score matmuls for kts 4b..4b+3.  Non-DIRECT
            kts are DVE-cast to pt; DIRECT kts leave psum tiles for ACT."""
            for kt in range(4 * b, 4 * b + 4):
                for ht in range(2):
                    ps = ps_s.tile([128, 1024], F32, tag="s", name="s_ps")
                    for j in range(2):
                        nc.tensor.matmul(
                            ps[:, j * 512:(j + 1) * 512],
                            kt_sb[64 * j:64 * j + 64, ht,
                                  kt * 128:(kt + 1) * 128],
                            qt_sb[64 * j:64 * j + 64, ht,
                                  c * 512:(c + 1) * 512],
                            start=True, stop=True)
                    if kt in DIRECT:
                        direct_ps[(kt, ht)] = ps
                    else:
                        nc.vector.tensor_copy(
                            pt_sb[:, kt, 2 * ht:2 * ht + 2, :],
                            ps.rearrange("p (h q) -> p h q", h=2))

        def exp_block(b, direct_ps):
            """ACT exp for kts 4b..4b+3: big in-place SBUF calls for the
            DVE-cast pairs, direct PSUM->SBUF calls for DIRECT kts."""
            for kt0 in (4 * b, 4 * b + 2):
                if kt0 in DIRECT:
                    for kt in (kt0, kt0 + 1):
                        for ht in range(2):
                            ps = direct_ps.pop((kt, ht))
                            nc.scalar.activation(
                                pt_sb[:, kt, 2 * ht:2 * ht + 2, :],
                                ps.rearrange("p (h q) -> p h q", h=2),
                                mybir.ActivationFunctionType.Exp, scale=0.125)
                else:
                    sl = pt_sb[:, kt0:kt0 + 2, :, :]
                    nc.scalar.activation(sl, sl,
                                         mybir.ActivationFunctionType.Exp,
                                         scale=0.125)

        def denom_block(ps, b):
            """Col-tiled (128x32) 4-head denominator accumulation."""
            for kt in range(4 * b, 4 * b + 4):
                for h in range(HPC):
                    nc.tensor.matmul(ps[32 * h:32 * h + 1, :], ones_sb,
                                     pt_sb[:, kt, h, :],
                                     start=(kt == 0), stop=(kt == KT - 1),
                                     skip_group_check=True,
                                     tile_position=(0, 32 * h))

        def ctx_block(ctx_ps, b):
            """Col-tiled (128x64) 2-head ctx accumulation for kts of b."""
            for kt in range(4 * b, 4 * b + 4):
                for h in range(HPC):
                    ht, j = h // 2, h % 2
                    nc.tensor.matmul(ctx_ps[ht][64 * j:64 * j + 64, :],
                                     vp_sb[:, kt, h, :], pt_sb[:, kt, h, :],
                                     start=(kt == 0), stop=(kt == KT - 1),
                                     skip_group_check=True)

        def finish_chunk(c, d_ps, ctx_ps):
            """reciprocal of denominators, bcast via selector matmul, then
            normalize ctx into ctxT."""
            nc.vector.reciprocal(recip_sb, d_ps)
            for ht in range(2):
                bc = ps_b.tile([128, 512], F32, tag="bc")
                nc.tensor.matmul(bc, E_sb[:, ht, :], recip_sb,
                                 start=True, stop=True)
                bc_sb = misc.tile([128, 512], F32, tag="bc_sb")
                nc.vector.tensor_copy(bc_sb, bc)
                nc.vector.tensor_mul(
                    ctxT_sb[:, ht, c * 512:(c + 1) * 512], ctx_ps[ht], bc_sb)

        def out_proj(c):
            """out[tok, :] = ctx^T_chunk @ Wo -> HBM (f32 partials)."""
            for t in range(4):
                tok = c * 512 + t * 128
                o_sb = outp.tile([128, D], F32, tag="o")
                pss = [ps_c.tile([128, 512], F32, tag="pc", name=f"o_ps{n}")
                       for n in range(2)]
                for k in range(2):
                    for n in range(2):
                        nc.tensor.matmul(
                            pss[n], ctxT_sb[:, k, tok:tok + 128],
                            wo_sb[:, k, n * 512:(n + 1) * 512],
                            start=(k == 0), stop=(k == 1))
                for n in range(2):
                    nc.vector.tensor_copy(o_sb[:, n * 512:(n + 1) * 512],
                                          pss[n])
                nc.sync.dma_start(out=d_out[tok:tok + 128, :], in_=o_sb)

        # ---- emission schedule (software-pipelined) -----------------------
        # ACT (exp) is the throughput bound; every cadence keeps it fed by
        # interleaving next-chunk scores into current-chunk consumption.
        direct_ps = {}
        proj_T(d_xqT, wq_sb, bq_sb, qt_sb, 0)
        load_late_weights()
        for b in range(4):
            proj_T(d_xkT, wk_sb, bk_sb, kt_sb, b)
            scores_block(0, b, direct_ps)
            exp_block(b, direct_ps)
        proj_T(d_xqT, wq_sb, bq_sb, qt_sb, 1)

        for c in range(NCH):
            d_ps = ps_d.tile([128, 512], F32, tag="d")
            nc.vector.memset(d_ps, 1.0)  # reciprocal background
            ctx_ps = [ps_c.tile([128, 512], F32, tag="pc", name=f"ctx{i}")
                      for i in range(2)]
            for b in range(4):
                if c == 0:
                    proj_V(b)
                denom_block(d_ps, b)
                ctx_block(ctx_ps, b)
                if c + 1 < NCH:
                    scores_block(c + 1, b, direct_ps)
                    exp_block(b, direct_ps)
            finish_chunk(c, d_ps, ctx_ps)
            if c + 2 < NCH:
                proj_T(d_xqT, wq_sb, bq_sb, qt_sb, c + 2)
            out_proj(c)

    _split_excess_waits(nc)
    return nc


_NC = None


def _get_nc():
    global _NC
    if _NC is None:
        _NC = _build()
    return _NC


def _make_in_maps(inputs):
    query = np.asarray(inputs["query"], np.float32)
    key = np.asarray(inputs["key"], np.float32)
    value = np.asarray(inputs["value"], np.float32)
    Wq, Wk, Wv, Wo = (np.asarray(inputs[a], np.float32)
                      for a in ("Wq", "Wk", "Wv", "Wo"))
    bq, bk = (np.asarray(inputs[a], np.float32) for a in ("bq", "bk"))

    in_maps = []
    for c in range(N_CORES):
        b, g = divmod(c, HPC)
        sl = slice(g * GD, (g + 1) * GD)
        in_maps.append({
            "xqT": np.ascontiguousarray(query[b].T.astype(np.float16)),
            "xkT": np.ascontiguousarray(key[b].T.astype(np.float16)),
            "xvT": np.ascontiguousarray(value[b].T.astype(np.float16)),
            "wq": np.ascontiguousarray(Wq[:, sl].astype(np.float16)),
            "wk": np.ascontiguousarray(Wk[:, sl].astype(np.float16)),
            "wv": np.ascontiguousarray(Wv[:, sl].astype(np.float16)),
            "wo": np.ascontiguousarray(Wo[sl, :].astype(np.float16)),
            "bq": np.ascontiguousarray(bq[sl]),
            "bk": np.ascontiguousarray(bk[sl]),
        })
    return in_maps


def kernel(query, key, value, Wq, bq, Wk, bk, Wv, bv, Wo, bo):
    bo = np.asarray(bo, np.float32)
    bv = np.asarray(bv, np.float32)
    Wo_f = np.asarray(Wo, np.float32)
    in_maps = _make_in_maps(dict(query=query, key=key, value=value,
                                 Wq=Wq, bq=bq, Wk=Wk, bk=bk, Wv=Wv,
                                 Wo=Wo))

    res = run_bass_kernel_spmd(_get_nc(), in_maps, list(range(N_CORES)))
    outs = [res.results[c]["out"] for c in range(N_CORES)]
    bias = bo + bv @ Wo_f  # bv enters ctx additively (softmax weights sum to 1)
    full = np.stack([
        outs[0] + outs[1] + outs[2] + outs[3],
        outs[4] + outs[5] + outs[6] + outs[7],
    ]).astype(np.float32)
    return full + bias


# revision 32
# speedup vs baseline: 1.0069x; 1.0049x over previous
"""MultiHeadAttention TRN2 kernel v2: B=2, S=2048, D=1024, H=16, Dh=64.

Sharding (8 cores): core c -> batch b=c//4, head-group g=c%4 (4 heads = 2
head-pairs, 256 model dims).  Tensor-parallel QKV + row-parallel output
projection; 4-way partial-output sum + bo + bv@Wo on host.

fp16 on-chip pipeline (host-sim rel err 1.2e-3):
  Q^T,K^T [128,2,2048] fp16   (dh+64*(h%2) on partitions, pair ht on free)
  V       [128,kt,h,64] fp16  natural (token on partitions)
  scores  s^T[k,q]: 2-head ROW-TILED matmul pairs (64x128 tiles), psum f32
  pt      fp16 raw scores staged to SBUF (DVE evac), exp'd IN PLACE by ACT
          in [128,4096] calls (scale=0.125)
  denom   4-head COL-TILED (128x32) ones-matmuls -> psum partitions 32h
  ctx     2-head COL-TILED (128x64) matmuls, accum over kt
  recip   DVE reciprocal on full [128,512] psum (memset-1.0 background)
  bcast   selector-matmul E^T @ recip -> per-q recip rows, DVE normalize
  out     ctx^T @ Wo natural, f32 partials to HBM

Emission is software-pipelined so ACT (the bottleneck: 16.8M exps/core)
never starves: fill = K-proj, Q0-proj, S(0), V-proj, exp(0), Q1-proj; then
per chunk: D, C, bcast+norm, Q-proj(c+2), O-proj(c), S(c+1), exp(c+1).
"""

import numpy as np

import concourse.bass as bass
import concourse.mybir as mybir
import concourse.tile as tile_mod
from concourse.tile import TileContext
from concourse.bass_utils import run_bass_kernel_spmd
from concourse.vector_clock import ScopedClock

# ---------------------------------------------------------------- drain patch
# This walrus build's TPB_CTRL drain lowering accepts only ONE sync wait per
# instruction; TileContext's tail drain carries one wait per live semaphore.
# Split it into a chain of drains with <=1 wait each.
_MAXW = 1


def _patched_drain_and_barrier(self, tick_clock, wait_clock):
    nc = self.nc
    drain_inst = nc.sync.drain()
    wait_clock.add_sem_waits(
        drain_inst.ins, ScopedClock({None: tick_clock.global_clock})
    )
    si = drain_inst.ins.sync_info
    if si is not None and si.on_wait and len(si.on_wait) > _MAXW:
        waits = list(si.on_wait)
        del si.on_wait[_MAXW:]
        for i in range(_MAXW, len(waits), _MAXW):
            d2 = nc.sync.drain()
            si2 = d2.ins.sync_info
            if si2 is None:
                d2.ins.sync_info = mybir.SyncInfo(on_wait=[], on_update=[])
                si2 = d2.ins.sync_info
            si2.on_wait.extend(waits[i : i + _MAXW])
    nc.all_engine_barrier()
    assert self.sems is not None
    popped = nc._tile_sem_poison_stack.pop()
    assert popped is self._sem_poison
    nc.clear_and_free_semaphores(list(self.sems.allocated().values()))
    nc.all_engine_barrier()


tile_mod.TileContext._drain_and_barrier = _patched_drain_and_barrier

# ---------------------------------------------------------------- constants
B, S, D = 2, 2048, 1024
H, DH = 16, 64
N_CORES = 8
HPC = 4          # heads per core (2 pairs)
GD = HPC * DH    # 256 model dims per core
KT = S // 128    # 16 key tiles
NCH = 4          # q chunks of 512
F32 = mybir.dt.float32
F16 = mybir.dt.float16


def _split_excess_waits(nc):
    """This walrus build accepts only ONE sync wait per instruction (any
    type).  Hoist extra waits onto same-engine nops inserted right before
    the over-subscribed instruction."""
    for fn in nc.m.functions:
        for bb in fn.blocks:
            insts = bb.instructions
            i = 0
            while i < len(insts):
                inst = insts[i]
                si = getattr(inst, "sync_info", None)
                if si is not None and si.on_wait and len(si.on_wait) > 1:
                    extra = list(si.on_wait[:-1])
                    del si.on_wait[:-1]
                    nops = []
                    for w in extra:
                        bi = nc.engines[inst.engine].nop(nofuse=True,
                                                         hint="waitsplit")
                        bi.ins.sync_info = mybir.SyncInfo(on_wait=[w],
                                                          on_update=[])
                        nops.append(bi.ins)
                    for ni in nops:
                        for fb in fn.blocks:
                            if ni in fb.instructions:
                                fb.instructions.remove(ni)
                                break
                    insts[i:i] = nops
                    i += len(nops)
                i += 1


def _build():
    from contextlib import ExitStack

    nc = bass.Bass("TRN2", target_bir_lowering=False, debug=False,
                   num_devices=N_CORES)
    d_xqT = nc.dram_tensor("xqT", [D, S], F16, kind="ExternalInput").ap()
    d_xkT = nc.dram_tensor("xkT", [D, S], F16, kind="ExternalInput").ap()
    d_xvT = nc.dram_tensor("xvT", [D, S], F16, kind="ExternalInput").ap()
    d_wq = nc.dram_tensor("wq", [D, GD], F16, kind="ExternalInput").ap()
    d_wk = nc.dram_tensor("wk", [D, GD], F16, kind="ExternalInput").ap()
    d_wv = nc.dram_tensor("wv", [D, GD], F16, kind="ExternalInput").ap()
    d_wo = nc.dram_tensor("wo", [GD, D], F16, kind="ExternalInput").ap()
    d_bq = nc.dram_tensor("bq", [GD], F32, kind="ExternalInput").ap()
    d_bk = nc.dram_tensor("bk", [GD], F32, kind="ExternalInput").ap()
    d_out = nc.dram_tensor("out", [S, D], F32, kind="ExternalOutput").ap()

    with TileContext(nc) as tc, ExitStack() as ctx:
        ctx.enter_context(nc.allow_low_precision(
            reason="fp16 on-chip pipeline; accumulation stays fp32 in PSUM"))
        wp = ctx.enter_context(tc.tile_pool(name="w", bufs=1))
        xp = ctx.enter_context(tc.tile_pool(name="x", bufs=3))
        big = ctx.enter_context(tc.tile_pool(name="big", bufs=1))
        outp = ctx.enter_context(tc.tile_pool(name="outp", bufs=2))
        misc = ctx.enter_context(tc.tile_pool(name="misc", bufs=2))
        ps_s = ctx.enter_context(
            tc.tile_pool(name="ps_s", bufs=2, space="PSUM"))
        ps_c = ctx.enter_context(
            tc.tile_pool(name="ps_c", bufs=2, space="PSUM"))
        ps_d = ctx.enter_context(
            tc.tile_pool(name="ps_d", bufs=1, space="PSUM"))
        ps_b = ctx.enter_context(
            tc.tile_pool(name="ps_b", bufs=1, space="PSUM"))

        # ---- weights to SBUF (k-tiled layouts)
        wq_sb = wp.tile([128, 8, GD], F16, tag="wq")
        nc.sync.dma_start(out=wq_sb, in_=d_wq.rearrange("(k p) n -> p k n", p=128))
        wk_sb = wp.tile([128, 8, GD], F16, tag="wk")
        nc.sync.dma_start(out=wk_sb, in_=d_wk.rearrange("(k p) n -> p k n", p=128))
        wv_sb = wp.tile([128, 8, GD], F16, tag="wv")
        nc.sync.dma_start(out=wv_sb, in_=d_wv.rearrange("(k p) n -> p k n", p=128))
        wo_sb = wp.tile([128, 2, D], F16, tag="wo")
        nc.sync.dma_start(out=wo_sb, in_=d_wo.rearrange("(k p) n -> p k n", p=128))
        bq_sb = wp.tile([128, 2], F32, tag="bq")
        nc.sync.dma_start(out=bq_sb, in_=d_bq.rearrange("(m p) -> p m", p=128))
        bk_sb = wp.tile([128, 2], F32, tag="bk")
        nc.sync.dma_start(out=bk_sb, in_=d_bk.rearrange("(m p) -> p m", p=128))

        # ones column for denominator matmuls
        ones_sb = wp.tile([128, 1], F16, tag="ones")
        nc.vector.memset(ones_sb, 1.0)
        # bcast selector: E[:, p, :]^T @ recip replicates denominator-recip
        # rows {64j} -> partition blocks [64j:64j+64) for pair p's heads.
        E_sb = wp.tile([128, 2, 128], F16, tag="E")
        nc.vector.memset(E_sb, 0.0)
        for p in range(2):
            for j in range(2):
                nc.vector.memset(E_sb[32 * (2 * p + j):32 * (2 * p + j) + 1,
                                      p, 64 * j:64 * j + 64], 1.0)

        qt_sb = big.tile([128, 2, S], F16, tag="qt")
        kt_sb = big.tile([128, 2, S], F16, tag="kt")
        vp_sb = big.tile([128, KT, HPC, DH], F16, tag="vp")
        ctxT_sb = big.tile([128, 2, S], F16, tag="ctxT")
        pt_sb = big.tile([128, KT, HPC, 512], F16, tag="pt")
        recip_sb = big.tile([128, 512], F16, tag="recip")

        # ---- projection helpers -------------------------------------------
        def proj_T(d_x, w_sb, b_sb, dst, n):
            """(x @ W + b)^T for one 512-token chunk -> dst[:, :, n*512:...]"""
            xr = d_x.rearrange("(k p) q -> p k q", p=128)
            xb = xp.tile([128, 8, 512], F16, tag="xb")
            nc.sync.dma_start(out=xb, in_=xr[:, :, n * 512:(n + 1) * 512])
            for m in range(2):
                ps = ps_c.tile([128, 512], F32, tag="pc")
                for k in range(8):
                    nc.tensor.matmul(ps, w_sb[:, k, m * 128:(m + 1) * 128],
                                     xb[:, k, :], start=(k == 0), stop=(k == 7))
                nc.vector.tensor_scalar_add(
                    dst[:, m, n * 512:(n + 1) * 512], ps, b_sb[:, m:m + 1])

        def proj_V(n):
            """V natural [tok,256] for one 512-token chunk (no bias: bv@Wo
            is folded into the host-side output bias)."""
            xr = d_xvT.rearrange("(k p) q -> p k q", p=128)
            xb = xp.tile([128, 8, 512], F16, tag="xb")
            nc.sync.dma_start(out=xb, in_=xr[:, :, n * 512:(n + 1) * 512])
            for t in range(4):
                ps = ps_b.tile([128, 512], F32, tag="bc", name="v_ps")
                for k in range(8):
                    nc.tensor.matmul(ps[:, 0:GD],
                                     xb[:, k, t * 128:(t + 1) * 128],
                                     wv_sb[:, k, :],
                                     start=(k == 0), stop=(k == 7))
                nc.vector.tensor_copy(
                    vp_sb[:, n * 4 + t, :, :],
                    ps[:, 0:GD].rearrange("p (h d) -> p h d", h=HPC))

        # kt pairs whose exp runs on ACT directly from PSUM (skipping the
        # DVE cast) -- balances ACT vs DVE throughput per cadence.
        DIRECT = set(range(16))

        # ---- attention phase helpers --------------------------------------
        def scores_block(c, b, direct_ps):
            """Row-tiled 2-head score matmuls for kts 4b..4b+3.  Non-DIRECT
            kts are DVE-cast to pt; DIRECT kts leave psum tiles for ACT."""
            for kt in range(4 * b, 4 * b + 4):
                for ht in range(2):
                    ps = ps_s.tile([128, 1024], F32, tag="s", name="s_ps")
                    for j in range(2):
                        nc.tensor.matmul(
                            ps[:, j * 512:(j + 1) * 512],
                            kt_sb[64 * j:64 * j + 64, ht,
                                  kt * 128:(kt + 1) * 128],
                            qt_sb[64 * j:64 * j + 64, ht,
                                  c * 512:(c + 1) * 512],
                            start=True, stop=True)
                    if kt in DIRECT:
                        direct_ps[(kt, ht)] = ps
                    else:
                        nc.vector.tensor_copy(
                            pt_sb[:, kt, 2 * ht:2 * ht + 2, :],
                            ps.rearrange("p (h q) -> p h q", h=2))

        def exp_block(b, direct_ps):
            """ACT exp for kts 4b..4b+3: big in-place SBUF calls for the
            DVE-cast pairs, direct PSUM->SBUF calls for DIRECT kts."""
            for kt0 in (4 * b, 4 * b + 2):
                if kt0 in DIRECT:
                    for kt in (kt0, kt0 + 1):
                        for ht in range(2):
                            ps = direct_ps.pop((kt, ht))
                            nc.scalar.activation(
                                pt_sb[:, kt, 2 * ht:2 * ht + 2, :],
                                ps.rearrange("p (h q) -> p h q", h=2),
                                mybir.ActivationFunctionType.Exp, scale=0.125)
                else:
                    sl = pt_sb[:, kt0:kt0 + 2, :, :]
                    nc.scalar.activation(sl, sl,
                                         mybir.ActivationFunctionType.Exp,
                                         scale=0.125)

        def denom_block(ps, b):
            """Col-tiled (128x32) 4-head denominator accumulation."""
            for kt in range(4 * b, 4 * b + 4):
                for h in range(HPC):
                    nc.tensor.matmul(ps[32 * h:32 * h + 1, :], ones_sb,
                                     pt_sb[:, kt, h, :],
                                     start=(kt == 0), stop=(kt == KT - 1),
                                     skip_group_check=True,
                                     tile_position=(0, 32 * h))

        def ctx_block(ctx_ps, b):
            """Col-tiled (128x64) 2-head ctx accumulation for kts of b."""
            for kt in range(4 * b, 4 * b + 4):
                for h in range(HPC):
                    ht, j = h // 2, h % 2
                    nc.tensor.matmul(ctx_ps[ht][64 * j:64 * j + 64, :],
                                     vp_sb[:, kt, h, :], pt_sb[:, kt, h, :],
                                     start=(kt == 0), stop=(kt == KT - 1),
                                     skip_group_check=True)

        def finish_chunk(c, d_ps, ctx_ps):
            """reciprocal of denominators, bcast via selector matmul, then
            normalize ctx into ctxT."""
            nc.vector.reciprocal(recip_sb, d_ps)
            for ht in range(2):
                bc = ps_b.tile([128, 512], F32, tag="bc")
                nc.tensor.matmul(bc, E_sb[:, ht, :], recip_sb,
                                 start=True, stop=True)
                bc_sb = misc.tile([128, 512], F32, tag="bc_sb")
                nc.vector.tensor_copy(bc_sb, bc)
                nc.vector.tensor_mul(
                    ctxT_sb[:, ht, c * 512:(c + 1) * 512], ctx_ps[ht], bc_sb)

        def out_proj(c):
            """out[tok, :] = ctx^T_chunk @ Wo -> HBM (f32 partials)."""
            for t in range(4):
                tok = c * 512 + t * 128
                o_sb = outp.tile([128, D], F32, tag="o")
                pss = [ps_c.tile([128, 512], F32, tag="pc", name=f"o_ps{n}")
                       for n in range(2)]
                for k in range(2):
                    for n in range(2):
                        nc.tensor.matmul(
                            pss[n], ctxT_sb[:, k, tok:tok + 128],
                            wo_sb[:, k, n * 512:(n + 1) * 512],
                            start=(k == 0), stop=(k == 1))
                for n in range(2):
                    nc.vector.tensor_copy(o_sb[:, n * 512:(n + 1) * 512],
                                          pss[n])
                nc.sync.dma_start(out=d_out[tok:tok + 128, :], in_=o_sb)

        # ---- emission schedule (software-pipelined) -----------------------
        # ACT (exp) is the throughput bound; every cadence keeps it fed by
        # interleaving next-chunk scores into current-chunk consumption.
        direct_ps = {}
        proj_T(d_xqT, wq_sb, bq_sb, qt_sb, 0)
        for b in range(4):
            proj_T(d_xkT, wk_sb, bk_sb, kt_sb, b)
            scores_block(0, b, direct_ps)
            exp_block(b, direct_ps)
        proj_T(d_xqT, wq_sb, bq_sb, qt_sb, 1)

        for c in range(NCH):
            d_ps = ps_d.tile([128, 512], F32, tag="d")
            nc.vector.memset(d_ps, 1.0)  # reciprocal background
            ctx_ps = [ps_c.tile([128, 512], F32, tag="pc", name=f"ctx{i}")
                      for i in range(2)]
            for b in range(4):
                if c == 0:
                    proj_V(b)
                denom_block(d_ps, b)
                ctx_block(ctx_ps, b)
                if c + 1 < NCH:
                    scores_block(c + 1, b, direct_ps)
                    exp_block(b, direct_ps)
            finish_chunk(c, d_ps, ctx_ps)
            if c + 2 < NCH:
                proj_T(d_xqT, wq_sb, bq_sb, qt_sb, c + 2)
            out_proj(c)

    _split_excess_waits(nc)
    return nc


_NC = None


def _get_nc():
    global _NC
    if _NC is None:
        _NC = _build()
    return _NC


def _make_in_maps(inputs):
    query = np.asarray(inputs["query"], np.float32)
    key = np.asarray(inputs["key"], np.float32)
    value = np.asarray(inputs["value"], np.float32)
    Wq, Wk, Wv, Wo = (np.asarray(inputs[a], np.float32)
                      for a in ("Wq", "Wk", "Wv", "Wo"))
    bq, bk = (np.asarray(inputs[a], np.float32) for a in ("bq", "bk"))

    in_maps = []
    for c in range(N_CORES):
        b, g = divmod(c, HPC)
        sl = slice(g * GD, (g + 1) * GD)
        in_maps.append({
            "xqT": np.ascontiguousarray(query[b].T.astype(np.float16)),
            "xkT": np.ascontiguousarray(key[b].T.astype(np.float16)),
            "xvT": np.ascontiguousarray(value[b].T.astype(np.float16)),
            "wq": np.ascontiguousarray(Wq[:, sl].astype(np.float16)),
            "wk": np.ascontiguousarray(Wk[:, sl].astype(np.float16)),
            "wv": np.ascontiguousarray(Wv[:, sl].astype(np.float16)),
            "wo": np.ascontiguousarray(Wo[sl, :].astype(np.float16)),
            "bq": np.ascontiguousarray(bq[sl]),
            "bk": np.ascontiguousarray(bk[sl]),
        })
    return in_maps


def kernel(query, key, value, Wq, bq, Wk, bk, Wv, bv, Wo, bo):
    bo = np.asarray(bo, np.float32)
    bv = np.asarray(bv, np.float32)
    Wo_f = np.asarray(Wo, np.float32)
    in_maps = _make_in_maps(dict(query=query, key=key, value=value,
                                 Wq=Wq, bq=bq, Wk=Wk, bk=bk, Wv=Wv,
                                 Wo=Wo))

    res = run_bass_kernel_spmd(_get_nc(), in_maps, list(range(N_CORES)))
    outs = [res.results[c]["out"] for c in range(N_CORES)]
    bias = bo + bv @ Wo_f  # bv enters ctx additively (softmax weights sum to 1)
    full = np.stack([
        outs[0] + outs[1] + outs[2] + outs[3],
        outs[4] + outs[5] + outs[6] + outs[7],
    ]).astype(np.float32)
    return full + bias


# revision 37
# speedup vs baseline: 1.0321x; 1.0251x over previous
"""MultiHeadAttention TRN2 kernel: B=2, S=2048, D=1024, H=16, Dh=64.

Sharding (8 cores): core c -> batch b=c//4, head-group g=c%4 (4 heads = 2
head-pairs, 256 model dims).  Tensor-parallel QKV + row-parallel output
projection; 4-way partial-output sum + bo + bv@Wo on host (bv enters the
context additively because softmax weights sum to 1, so the device never
sees bv).

fp16 on-chip pipeline (HW rel err ~1.1e-3 vs 2e-2 budget):
  Q^T,K^T [128,2,2048] fp16   (dh+64*(h%2) on partitions, pair ht on free)
  V       [128,kt,h,64] fp16  natural (token on partitions)
  scores  s^T[k,q]: 2-head ROW-TILED matmul pairs (64x128 array tiles,
          concurrent), fp32 psum
  exp     ACT reads each [128,1024] score psum tile directly (Exp,
          scale=0.125) and writes fp16 P^T to SBUF -- the exp IS the psum
          evacuation; ACT (16.8M exps/core, ~135us) is the pacer and DVE
          stays off the critical path
  denom   4-head COL-TILED (128x32) ones-matmuls accumulate per-q sums on
          psum partitions 32h
  ctx     2-head COL-TILED (128x64) matmuls, accum over all 16 k-tiles
  recip   DVE reciprocal on the full [128,512] denom bank (memset-1.0
          background keeps garbage partitions finite)
  bcast   selector-matmul E^T @ recip broadcasts per-q reciprocals to all
          64 partitions of each head; DVE multiply normalizes into ctx^T
  out     ctx^T @ Wo natural, f32 partials DMA'd to HBM

Emission is software-pipelined per 4-kt block so ACT never starves:
fill = Q0-proj + [K-proj(b), scores(0,b), exp(0,b)] x4 + Q1-proj; then per
chunk c: [V-proj(b) (c=0 only), denom(c,b), ctx(c,b), scores(c+1,b),
exp(c+1,b)] x4, finish (recip/bcast/normalize), Q-proj(c+2), out-proj(c).
Scores for chunk c+1 overwrite pt only after denom/ctx of chunk c read it
(same-block WAR ordering via Tile semaphores).
"""

import numpy as np

import concourse.bass as bass
import concourse.mybir as mybir
import concourse.tile as tile_mod
from concourse.tile import TileContext
from concourse.bass_utils import run_bass_kernel_spmd
from concourse.vector_clock import ScopedClock

# ---------------------------------------------------------------- drain patch
# This walrus build's TPB_CTRL drain lowering accepts only ONE sync wait per
# instruction; TileContext's tail drain carries one wait per live semaphore.
# Split it into a chain of drains with <=1 wait each.
_MAXW = 1


def _patched_drain_and_barrier(self, tick_clock, wait_clock):
    nc = self.nc
    drain_inst = nc.sync.drain()
    wait_clock.add_sem_waits(
        drain_inst.ins, ScopedClock({None: tick_clock.global_clock})
    )
    si = drain_inst.ins.sync_info
    if si is not None and si.on_wait and len(si.on_wait) > _MAXW:
        waits = list(si.on_wait)
        del si.on_wait[_MAXW:]
        for i in range(_MAXW, len(waits), _MAXW):
            d2 = nc.sync.drain()
            si2 = d2.ins.sync_info
            if si2 is None:
                d2.ins.sync_info = mybir.SyncInfo(on_wait=[], on_update=[])
                si2 = d2.ins.sync_info
            si2.on_wait.extend(waits[i : i + _MAXW])
    nc.all_engine_barrier()
    assert self.sems is not None
    popped = nc._tile_sem_poison_stack.pop()
    assert popped is self._sem_poison
    nc.clear_and_free_semaphores(list(self.sems.allocated().values()))
    nc.all_engine_barrier()


tile_mod.TileContext._drain_and_barrier = _patched_drain_and_barrier

# ---------------------------------------------------------------- constants
B, S, D = 2, 2048, 1024
H, DH = 16, 64
N_CORES = 8
HPC = 4          # heads per core (2 pairs)
GD = HPC * DH    # 256 model dims per core
KT = S // 128    # 16 key tiles
NCH = 4          # q chunks of 512
F32 = mybir.dt.float32
F16 = mybir.dt.float16


def _split_excess_waits(nc):
    """This walrus build accepts only ONE sync wait per instruction (any
    type).  Hoist extra waits onto same-engine nops inserted right before
    the over-subscribed instruction."""
    for fn in nc.m.functions:
        for bb in fn.blocks:
            insts = bb.instructions
            i = 0
            while i < len(insts):
                inst = insts[i]
                si = getattr(inst, "sync_info", None)
                if si is not None and si.on_wait and len(si.on_wait) > 1:
                    extra = list(si.on_wait[:-1])
                    del si.on_wait[:-1]
                    nops = []
                    for w in extra:
                        bi = nc.engines[inst.engine].nop(nofuse=True,
                                                         hint="waitsplit")
                        bi.ins.sync_info = mybir.SyncInfo(on_wait=[w],
                                                          on_update=[])
                        nops.append(bi.ins)
                    for ni in nops:
                        for fb in fn.blocks:
                            if ni in fb.instructions:
                                fb.instructions.remove(ni)
                                break
                    insts[i:i] = nops
                    i += len(nops)
                i += 1


def _build():
    from contextlib import ExitStack

    nc = bass.Bass("TRN2", target_bir_lowering=False, debug=False,
                   num_devices=N_CORES)
    d_xqT = nc.dram_tensor("xqT", [D, S], F16, kind="ExternalInput").ap()
    d_xkT = nc.dram_tensor("xkT", [D, S], F16, kind="ExternalInput").ap()
    d_xvT = nc.dram_tensor("xvT", [D, S], F16, kind="ExternalInput").ap()
    d_wq = nc.dram_tensor("wq", [D, GD], F16, kind="ExternalInput").ap()
    d_wk = nc.dram_tensor("wk", [D, GD], F16, kind="ExternalInput").ap()
    d_wv = nc.dram_tensor("wv", [D, GD], F16, kind="ExternalInput").ap()
    d_wo = nc.dram_tensor("wo", [GD, D], F16, kind="ExternalInput").ap()
    d_bq = nc.dram_tensor("bq", [GD], F32, kind="ExternalInput").ap()
    d_bk = nc.dram_tensor("bk", [GD], F32, kind="ExternalInput").ap()
    d_out = nc.dram_tensor("out", [S, D], F16, kind="ExternalOutput").ap()

    with TileContext(nc) as tc, ExitStack() as ctx:
        ctx.enter_context(nc.allow_low_precision(
            reason="fp16 on-chip pipeline; accumulation stays fp32 in PSUM"))
        wp = ctx.enter_context(tc.tile_pool(name="w", bufs=1))
        xp = ctx.enter_context(tc.tile_pool(name="x", bufs=3))
        big = ctx.enter_context(tc.tile_pool(name="big", bufs=1))
        outp = ctx.enter_context(tc.tile_pool(name="outp", bufs=2))
        misc = ctx.enter_context(tc.tile_pool(name="misc", bufs=2))
        ps_s = ctx.enter_context(
            tc.tile_pool(name="ps_s", bufs=2, space="PSUM"))
        ps_c = ctx.enter_context(
            tc.tile_pool(name="ps_c", bufs=2, space="PSUM"))
        ps_d = ctx.enter_context(
            tc.tile_pool(name="ps_d", bufs=1, space="PSUM"))
        ps_b = ctx.enter_context(
            tc.tile_pool(name="ps_b", bufs=1, space="PSUM"))

        # ---- weights to SBUF (k-tiled layouts)
        wq_sb = wp.tile([128, 8, GD], F16, tag="wq")
        nc.sync.dma_start(out=wq_sb, in_=d_wq.rearrange("(k p) n -> p k n", p=128))
        wk_sb = wp.tile([128, 8, GD], F16, tag="wk")
        nc.sync.dma_start(out=wk_sb, in_=d_wk.rearrange("(k p) n -> p k n", p=128))
        wv_sb = wp.tile([128, 8, GD], F16, tag="wv")
        nc.sync.dma_start(out=wv_sb, in_=d_wv.rearrange("(k p) n -> p k n", p=128))
        wo_sb = wp.tile([128, 2, D], F16, tag="wo")
        nc.sync.dma_start(out=wo_sb, in_=d_wo.rearrange("(k p) n -> p k n", p=128))
        bq_sb = wp.tile([128, 2], F32, tag="bq")
        nc.sync.dma_start(out=bq_sb, in_=d_bq.rearrange("(m p) -> p m", p=128))
        bk_sb = wp.tile([128, 2], F32, tag="bk")
        nc.sync.dma_start(out=bk_sb, in_=d_bk.rearrange("(m p) -> p m", p=128))

        # ones column for denominator matmuls
        ones_sb = wp.tile([128, 1], F16, tag="ones")
        nc.vector.memset(ones_sb, 1.0)
        # bcast selector: E[:, p, :]^T @ recip replicates denominator-recip
        # rows {64j} -> partition blocks [64j:64j+64) for pair p's heads.
        E_sb = wp.tile([128, 2, 128], F16, tag="E")
        nc.vector.memset(E_sb, 0.0)
        for p in range(2):
            for j in range(2):
                nc.vector.memset(E_sb[32 * (2 * p + j):32 * (2 * p + j) + 1,
                                      p, 64 * j:64 * j + 64], 1.0)

        qt_sb = big.tile([128, 2, S], F16, tag="qt")
        kt_sb = big.tile([128, 2, S], F16, tag="kt")
        vp_sb = big.tile([128, KT, HPC, DH], F16, tag="vp")
        ctxT_sb = big.tile([128, 2, S], F16, tag="ctxT")
        pt_sb = big.tile([128, KT, HPC, 512], F16, tag="pt")
        recip_sb = big.tile([128, 512], F16, tag="recip")

        # ---- projection helpers -------------------------------------------
        def proj_T(d_x, w_sb, b_sb, dst, n):
            """(x @ W + b)^T for one 512-token chunk -> dst[:, :, n*512:...]"""
            xr = d_x.rearrange("(k p) q -> p k q", p=128)
            xb = xp.tile([128, 8, 512], F16, tag="xb")
            nc.sync.dma_start(out=xb, in_=xr[:, :, n * 512:(n + 1) * 512])
            for m in range(2):
                ps = ps_c.tile([128, 512], F32, tag="pc")
                for k in range(8):
                    nc.tensor.matmul(ps, w_sb[:, k, m * 128:(m + 1) * 128],
                                     xb[:, k, :], start=(k == 0), stop=(k == 7))
                nc.vector.tensor_scalar_add(
                    dst[:, m, n * 512:(n + 1) * 512], ps, b_sb[:, m:m + 1])

        def proj_V(n):
            """V natural [tok,256] for one 512-token chunk (no bias: bv@Wo
            is folded into the host-side output bias)."""
            xr = d_xvT.rearrange("(k p) q -> p k q", p=128)
            xb = xp.tile([128, 8, 512], F16, tag="xb")
            nc.sync.dma_start(out=xb, in_=xr[:, :, n * 512:(n + 1) * 512])
            for t in range(4):
                ps = ps_b.tile([128, 512], F32, tag="bc", name="v_ps")
                for k in range(8):
                    nc.tensor.matmul(ps[:, 0:GD],
                                     xb[:, k, t * 128:(t + 1) * 128],
                                     wv_sb[:, k, :],
                                     start=(k == 0), stop=(k == 7))
                nc.vector.tensor_copy(
                    vp_sb[:, n * 4 + t, :, :],
                    ps[:, 0:GD].rearrange("p (h d) -> p h d", h=HPC))

        # kt pairs whose exp runs on ACT directly from PSUM (skipping the
        # DVE cast) -- balances ACT vs DVE throughput per cadence.
        DIRECT = set(range(16))

        # ---- attention phase helpers --------------------------------------
        def scores_block(c, b, direct_ps):
            """Row-tiled 2-head score matmuls for kts 4b..4b+3.  Non-DIRECT
            kts are DVE-cast to pt; DIRECT kts leave psum tiles for ACT."""
            for kt in range(4 * b, 4 * b + 4):
                for ht in range(2):
                    ps = ps_s.tile([128, 1024], F32, tag="s", name="s_ps")
                    for j in range(2):
                        nc.tensor.matmul(
                            ps[:, j * 512:(j + 1) * 512],
                            kt_sb[64 * j:64 * j + 64, ht,
                                  kt * 128:(kt + 1) * 128],
                            qt_sb[64 * j:64 * j + 64, ht,
                                  c * 512:(c + 1) * 512],
                            start=True, stop=True)
                    if kt in DIRECT:
                        direct_ps[(kt, ht)] = ps
                    else:
                        nc.vector.tensor_copy(
                            pt_sb[:, kt, 2 * ht:2 * ht + 2, :],
                            ps.rearrange("p (h q) -> p h q", h=2))

        def exp_block(b, direct_ps):
            """ACT exp for kts 4b..4b+3: big in-place SBUF calls for the
            DVE-cast pairs, direct PSUM->SBUF calls for DIRECT kts."""
            for kt0 in (4 * b, 4 * b + 2):
                if kt0 in DIRECT:
                    for kt in (kt0, kt0 + 1):
                        for ht in range(2):
                            ps = direct_ps.pop((kt, ht))
                            nc.scalar.activation(
                                pt_sb[:, kt, 2 * ht:2 * ht + 2, :],
                                ps.rearrange("p (h q) -> p h q", h=2),
                                mybir.ActivationFunctionType.Exp, scale=0.125)
                else:
                    sl = pt_sb[:, kt0:kt0 + 2, :, :]
                    nc.scalar.activation(sl, sl,
                                         mybir.ActivationFunctionType.Exp,
                                         scale=0.125)

        def denom_block(ps, b):
            """Col-tiled (128x32) 4-head denominator accumulation."""
            for kt in range(4 * b, 4 * b + 4):
                for h in range(HPC):
                    nc.tensor.matmul(ps[32 * h:32 * h + 1, :], ones_sb,
                                     pt_sb[:, kt, h, :],
                                     start=(kt == 0), stop=(kt == KT - 1),
                                     skip_group_check=True,
                                     tile_position=(0, 32 * h))

        def ctx_block(ctx_ps, b):
            """Col-tiled (128x64) 2-head ctx accumulation for kts of b."""
            for kt in range(4 * b, 4 * b + 4):
                for h in range(HPC):
                    ht, j = h // 2, h % 2
                    nc.tensor.matmul(ctx_ps[ht][64 * j:64 * j + 64, :],
                                     vp_sb[:, kt, h, :], pt_sb[:, kt, h, :],
                                     start=(kt == 0), stop=(kt == KT - 1),
                                     skip_group_check=True)

        def finish_chunk(c, d_ps, ctx_ps):
            """reciprocal of denominators, bcast via selector matmul, then
            normalize ctx into ctxT."""
            nc.vector.reciprocal(recip_sb, d_ps)
            for ht in range(2):
                bc = ps_b.tile([128, 512], F32, tag="bc")
                nc.tensor.matmul(bc, E_sb[:, ht, :], recip_sb,
                                 start=True, stop=True)
                bc_sb = misc.tile([128, 512], F32, tag="bc_sb")
                nc.vector.tensor_copy(bc_sb, bc)
                nc.vector.tensor_mul(
                    ctxT_sb[:, ht, c * 512:(c + 1) * 512], ctx_ps[ht], bc_sb)

        def out_proj(c):
            """out[tok, :] = ctx^T_chunk @ Wo -> HBM (f32 partials)."""
            for t in range(4):
                tok = c * 512 + t * 128
                o_sb = outp.tile([128, D], F16, tag="o")
                pss = [ps_c.tile([128, 512], F32, tag="pc", name=f"o_ps{n}")
                       for n in range(2)]
                for k in range(2):
                    for n in range(2):
                        nc.tensor.matmul(
                            pss[n], ctxT_sb[:, k, tok:tok + 128],
                            wo_sb[:, k, n * 512:(n + 1) * 512],
                            start=(k == 0), stop=(k == 1))
                for n in range(2):
                    nc.vector.tensor_copy(o_sb[:, n * 512:(n + 1) * 512],
                                          pss[n])
                nc.sync.dma_start(out=d_out[tok:tok + 128, :], in_=o_sb)

        def out_proj_t_ring(c, t):
            """One 128-token out tile via the score psum ring (used inside
            cadence-3 blocks while ctx accumulators occupy ps_c)."""
            tok = c * 512 + t * 128
            o_sb = outp.tile([128, D], F16, tag="o", name="o_sb_r")
            ps = ps_s.tile([128, 1024], F32, tag="s", name="o_ps_r")
            for k in range(2):
                for n in range(2):
                    nc.tensor.matmul(
                        ps[:, n * 512:(n + 1) * 512],
                        ctxT_sb[:, k, tok:tok + 128],
                        wo_sb[:, k, n * 512:(n + 1) * 512],
                        start=(k == 0), stop=(k == 1))
            nc.vector.tensor_copy(o_sb, ps)
            nc.sync.dma_start(out=d_out[tok:tok + 128, :], in_=o_sb)

        # ---- emission schedule (software-pipelined) -----------------------
        # ACT (exp) is the throughput bound; every cadence keeps it fed by
        # interleaving next-chunk scores into current-chunk consumption.
        direct_ps = {}
        proj_T(d_xqT, wq_sb, bq_sb, qt_sb, 0)
        for b in range(4):
            proj_T(d_xkT, wk_sb, bk_sb, kt_sb, b)
            scores_block(0, b, direct_ps)
            exp_block(b, direct_ps)
        proj_T(d_xqT, wq_sb, bq_sb, qt_sb, 1)

        for c in range(NCH):
            d_ps = ps_d.tile([128, 512], F32, tag="d")
            nc.vector.memset(d_ps, 1.0)  # reciprocal background
            ctx_ps = [ps_c.tile([128, 512], F32, tag="pc", name=f"ctx{i}")
                      for i in range(2)]
            for b in range(4):
                if c == 0:
                    proj_V(b)
                if c == NCH - 1:
                    out_proj_t_ring(NCH - 2, b)
                denom_block(d_ps, b)
                ctx_block(ctx_ps, b)
                if c + 1 < NCH:
                    scores_block(c + 1, b, direct_ps)
                    exp_block(b, direct_ps)
            finish_chunk(c, d_ps, ctx_ps)
            if c + 2 < NCH:
                proj_T(d_xqT, wq_sb, bq_sb, qt_sb, c + 2)
            if c != NCH - 2:
                out_proj(c)

    _split_excess_waits(nc)
    return nc


_NC = None


def _get_nc():
    global _NC
    if _NC is None:
        _NC = _build()
    return _NC


def _make_in_maps(inputs):
    query = np.asarray(inputs["query"], np.float32)
    key = np.asarray(inputs["key"], np.float32)
    value = np.asarray(inputs["value"], np.float32)
    Wq, Wk, Wv, Wo = (np.asarray(inputs[a], np.float32)
                      for a in ("Wq", "Wk", "Wv", "Wo"))
    bq, bk = (np.asarray(inputs[a], np.float32) for a in ("bq", "bk"))

    in_maps = []
    for c in range(N_CORES):
        b, g = divmod(c, HPC)
        sl = slice(g * GD, (g + 1) * GD)
        in_maps.append({
            "xqT": np.ascontiguousarray(query[b].T.astype(np.float16)),
            "xkT": np.ascontiguousarray(key[b].T.astype(np.float16)),
            "xvT": np.ascontiguousarray(value[b].T.astype(np.float16)),
            "wq": np.ascontiguousarray(Wq[:, sl].astype(np.float16)),
            "wk": np.ascontiguousarray(Wk[:, sl].astype(np.float16)),
            "wv": np.ascontiguousarray(Wv[:, sl].astype(np.float16)),
            "wo": np.ascontiguousarray(Wo[sl, :].astype(np.float16)),
            "bq": np.ascontiguousarray(bq[sl]),
            "bk": np.ascontiguousarray(bk[sl]),
        })
    return in_maps


def kernel(query, key, value, Wq, bq, Wk, bk, Wv, bv, Wo, bo):
    bo = np.asarray(bo, np.float32)
    bv = np.asarray(bv, np.float32)
    Wo_f = np.asarray(Wo, np.float32)
    in_maps = _make_in_maps(dict(query=query, key=key, value=value,
                                 Wq=Wq, bq=bq, Wk=Wk, bk=bk, Wv=Wv,
                                 Wo=Wo))

    res = run_bass_kernel_spmd(_get_nc(), in_maps, list(range(N_CORES)))
    outs = [np.asarray(res.results[c]["out"], np.float32)
            for c in range(N_CORES)]
    bias = bo + bv @ Wo_f  # bv enters ctx additively (softmax weights sum to 1)
    full = np.stack([
        outs[0] + outs[1] + outs[2] + outs[3],
        outs[4] + outs[5] + outs[6] + outs[7],
    ]).astype(np.float32)
    return full + bias
